# revision 52
# baseline (speedup 1.0000x reference)
"""Binary-weight 3x3 conv (depth-1 conv3d), 32ch -> 32ch, on trn2.

Forward pass of a BNN conv: effective weights are scale[o,i] * sign(w[o,i,kh,kw])
(the straight-through-estimator machinery in the reference only affects grads).
Kernel depth is 1, so this is a 2D 3x3 same-padded conv applied independently to
each of N*D = 8*16 = 128 images of shape [32, 160, 160].

Strategy (per core; batch dim sharded 1:1 onto 8 cores):
  - 16 d-slices per core, processed in 4 groups of 4 images.
  - Images live in SBUF zero-padded to 162 cols (host pre-pads + casts to
    bf16; rel-err budget 2e-2 >> bf16 conv error ~4e-3), on 32 channels =
    partitions [32r, 32r+32) for image r of the group.
  - PE runs in 32x32 tile-packing mode: tile (r, c) computes image r,
    pixel-segment c. 16 tile matmuls per tap, 9 taps accumulate in PSUM
    (tap shifts = free-axis offsets into the padded image).
  - bf16 matmuls (1 cyc/row); fp32 exact needs 2 passes and is ~2x slower.
    Measured floor: the PE moving-data path streams ~one 128-partition
    column/cycle, so a round (12 out rows x 4 images) costs ~9 taps x 4 segs
    x 480 cols ~ 6us regardless of instruction mix; the kernel sits on that.
  - PSUM evacuated to SBUF as bf16 split between DVE and ACT (halves the
    out DMA), then one DMA per seg writes into the NCDHW output layout;
    host casts the bf16 result back to f32.
"""

import numpy as np

import concourse.bass as bass
import concourse.mybir as mybir
import concourse.tile as tile
from concourse import bacc
from concourse import bass_utils
from concourse.tile_rust import add_dep_helper

C = 32          # in = out channels
KH = KW = 3

# full-problem dims
FULL_N, FULL_D, FULL_H, FULL_W = 8, 16, 160, 160


def build_conv(tc, out_ap, x_ap, w_ap, D, H, W, seg_rows, strip_rows, variant):
    """Emit the conv program for one core. x: [32, D, H, W], out: [32, D, H, W].

    variant "f32":   exact fp32 matmuls (4 cyc/row), w: [128, 288] f32.
    variant "bf16x3": x and w each split into bf16 hi+lo; accumulate
        w_hi*x_hi + w_hi*x_lo + w_lo*x_hi (error ~2^-18), w: [128, 2, 288] bf16.
    """
    nc = tc.nc
    f32 = mybir.dt.float32
    bf16 = mybir.dt.bfloat16
    mm_dt = f32 if variant == "f32" else bf16
    bf_like = variant in ("bf16", "bf16w", "bf16h")
    shared_w = variant == "bf16w"
    hsplit = variant == "bf16h"
    x_dt = bf16 if bf_like else f32
    st_dt = bf16 if bf_like else f32

    IPG = 4                      # images per group (row tiles)
    SEGS = 4                     # pixel segments per round (col tiles)
    NMM = seg_rows * W           # moving free size per matmul
    RPR = SEGS * seg_rows        # output rows per round
    assert D % IPG == 0
    NGRP = D // IPG
    WP = W + 2
    assert NMM <= 512

    # round origins (first output row of each round); if RPR doesn't divide
    # H, a final overlap round recomputes a few rows (identical values)
    origins = list(range(0, H - RPR + 1, RPR))
    covered = origins[-1] + RPR
    if covered < H:
        origins.append(H - RPR)
    new_from = {}
    for k, j0 in enumerate(origins):
        new_from[j0] = max(j0, origins[k - 1] + RPR) if k else j0
    rounds_per_strip = max(1, strip_rows // RPR)
    strips = [
        origins[i : i + rounds_per_strip]
        for i in range(0, len(origins), rounds_per_strip)
    ]
    XROWS = max(js[-1] + RPR + 2 - js[0] for js in strips)

    # x_ap is host-prepadded: [D, C, H+2, W+2] with zero borders, so a strip
    # is one fully-contiguous DMA per partition (partition stride = (H+2)*(W+2)).
    x_r = x_ap.rearrange("(g p) hp wp -> g p (hp wp)", g=NGRP, p=IPG * C)
    # [g] -> (o, r, h*w): row-sliced per round/seg at DMA time. o outermost
    # so the DGE splits each output DMA across all 16 SDMA engines (it
    # splits on the outermost dest dim; with 4 outermost it used only 4)
    out_v = out_ap.rearrange("o (g r) h w -> g o r (h w)", g=NGRP, r=IPG)

    xbytes = XROWS * WP * (2 if bf_like else 4)
    xbufs = 3 if (variant != "bf16x3" and 3 * xbytes < 160 * 1024) else 2
    with (
        tc.tile_pool(name="wpool", bufs=1) as wpool,
        tc.tile_pool(name="xpool", bufs=xbufs) as xpool,
        tc.tile_pool(name="stpool", bufs=3) as stpool,
        tc.tile_pool(name="pspool", bufs=2, space="PSUM") as pspool,
    ):
        if variant == "f32":
            w_sb = wpool.tile([128, KH * KW * C], f32, tag="w")
        elif variant in ("bf16", "bf16h"):
            w_sb = wpool.tile([128, KH * KW * C], bf16, tag="w")
        elif variant == "bf16w":
            w_sb = wpool.tile([128, KH * KW, 128], bf16, tag="w")
        else:
            w_sb = wpool.tile([128, 2, KH * KW * C], bf16, tag="w")
        nc.sync.dma_start(w_sb[:], w_ap[:])
        prev_mms = []  # tensor-engine MMs the next array reload must follow
        prev_h1 = {}   # bf16h: last weight-reusing MM per tile position

        for g in range(NGRP):
            for si, strip in enumerate(strips):
                X32 = xpool.tile([128, XROWS, WP], x_dt, tag="X32")
                r0 = strip[0]
                nrows = strip[-1] + RPR + 2 - r0
                # padded rows [r0, r0+nrows) of each image, contiguous runs
                # per partition. The very first strip is split into chunks so
                # the first rounds' matmuls start ~20us earlier.
                # chunk strip DMAs into ~2-round (~1MB) pieces so rounds wait
                # on partial arrivals and HBM load is smoothed; the first
                # strip's first chunk is one round so matmuls start early
                step = 2 * RPR
                cuts = ([0]
                        + [k * step + 2 for k in range(1, max(0, (nrows - 3))
                                                       // step + 1)]
                        + [nrows])
                if g == 0 and si == 0 and nrows > 3 * RPR:
                    cuts = sorted(set([RPR + 2] + cuts))
                cuts = sorted(set(c for c in cuts if c <= nrows))
                for a, b in zip(cuts, cuts[1:]):
                    nc.sync.dma_start(
                        X32[:, a:b, :].rearrange("p a b -> p (a b)"),
                        x_r[g][:, (r0 + a) * WP : (r0 + b) * WP],
                    )

                if variant in ("f32", "bf16", "bf16w", "bf16h"):
                    # comp -> (weight slice index or None, moving buffer)
                    comps = [(None, X32)]
                else:
                    Xhi = xpool.tile([128, XROWS, WP], bf16, tag="Xhi")
                    Xlo = xpool.tile([128, XROWS, WP], bf16, tag="Xlo")
                    nc.scalar.copy(Xhi[:, 0:nrows, :], X32[:, 0:nrows, :])
                    nc.vector.tensor_sub(
                        Xlo[:, 0:nrows, :], X32[:, 0:nrows, :],
                        Xhi[:, 0:nrows, :],
                    )
                    comps = [(0, Xhi), (0, Xlo), (1, Xhi)]

                for j0 in strip:
                    # split the round's accumulators into two 2-bank tiles so
                    # each half releases as soon as its own evac copy is done
                    psA = pspool.tile([128, 2, 512], f32, tag="psA")
                    psB = pspool.tile([128, 2, 512], f32, tag="psB")
                    for ci, (wi, XB) in enumerate(comps):
                        for tap in range(KH * KW):
                            kh, kw = divmod(tap, KW)
                            if shared_w:
                                # one 128-col array load for all 16 tiles of
                                # this tap (w replicated 4x4 host-side)
                                ldw = nc.tensor.ldweights(w_sb[:, tap, :])
                                for m in prev_mms:
                                    add_dep_helper(
                                        ldw.ins, m.ins, sync=False,
                                        reason="array reload after prior tap",
                                    )
                                prev_mms = []
                            if hsplit:
                                # split each tile matmul into a 1-row (h0,
                                # carries the LDW + bank-clear) and a 2-row
                                # (h1, ldweights=False) piece, h0s of all
                                # tiles first: doubles the per-lane tile
                                # handoffs whose fill/drain overlap (~100cyc
                                # each) shortens the serialized stream
                                hmms = {}
                                for h in (0, 1):
                                    for c in range(SEGS):
                                        for r in range(IPG):
                                            lhsT = w_sb[
                                                32 * r : 32 * r + 32,
                                                32 * tap : 32 * tap + 32,
                                            ]
                                            j = j0 - r0 + seg_rows * c
                                            psh = psA if r < 2 else psB
                                            if h == 0:
                                                mm = nc.tensor.matmul(
                                                    psh[32 * c : 32 * c + 32,
                                                        r % 2, 0:W],
                                                    lhsT,
                                                    XB[32 * r : 32 * r + 32,
                                                       j + kh : j + kh + 1,
                                                       kw : kw + W],
                                                    start=(tap == 0),
                                                    stop=(tap == KH * KW - 1),
                                                    tile_position=(32 * r,
                                                                   32 * c),
                                                )
                                                prev = prev_h1.pop((c, r), None)
                                                if prev is not None:
                                                    add_dep_helper(
                                                        mm.ins, prev.ins,
                                                        sync=False,
                                                        reason="h1 before reload",
                                                    )
                                                hmms[(c, r)] = mm
                                            else:
                                                mm = nc.tensor.matmul(
                                                    psh[32 * c : 32 * c + 32,
                                                        r % 2, W:NMM],
                                                    lhsT,
                                                    XB[32 * r : 32 * r + 32,
                                                       j + kh + 1 :
                                                       j + kh + seg_rows,
                                                       kw : kw + W],
                                                    start=False,
                                                    stop=(tap == KH * KW - 1),
                                                    tile_position=(32 * r,
                                                                   32 * c),
                                                    skip_group_check=True,
                                                )
                                                mm.ins.ldweights = False
                                                add_dep_helper(
                                                    mm.ins,
                                                    hmms[(c, r)].ins,
                                                    sync=False,
                                                    reason="reuse h0 weights",
                                                )
                                                prev_h1[(c, r)] = mm
                                continue
                            # r innermost: consecutive matmuls hit different
                            # PE row groups so their weight loads overlap
                            for c in range(SEGS):
                                for r in range(IPG):
                                    if shared_w:
                                        lhsT = w_sb[
                                            32 * r : 32 * r + 32, tap,
                                            32 * c : 32 * c + 32,
                                        ]
                                    elif wi is None:
                                        lhsT = w_sb[
                                            32 * r : 32 * r + 32,
                                            32 * tap : 32 * tap + 32,
                                        ]
                                    else:
                                        lhsT = w_sb[
                                            32 * r : 32 * r + 32, wi,
                                            32 * tap : 32 * tap + 32,
                                        ]
                                    j = j0 - r0 + seg_rows * c
                                    rhs = XB[
                                        32 * r : 32 * r + 32,
                                        j + kh : j + kh + seg_rows,
                                        kw : kw + W,
                                    ]
                                    psh = psA if r < 2 else psB
                                    mm = nc.tensor.matmul(
                                        psh[32 * c : 32 * c + 32, r % 2, 0:NMM],
                                        lhsT,
                                        rhs,
                                        start=(ci == 0 and tap == 0),
                                        stop=(
                                            ci == len(comps) - 1
                                            and tap == KH * KW - 1
                                        ),
                                        tile_position=(32 * r, 32 * c),
                                    )
                                    if shared_w:
                                        mm.ins.ldweights = False
                                        add_dep_helper(
                                            mm.ins, ldw.ins, sync=False,
                                            reason="use tap weights",
                                        )
                                        prev_mms.append(mm)
                    st = stpool.tile([128, SEGS, NMM], st_dt, tag="st")
                    nc.vector.tensor_copy(st[:, 0:2, :], psA[:, :, 0:NMM])
                    nc.scalar.copy(st[:, 2:4, :], psB[:, :, 0:NMM])
                    for c in range(SEGS):
                        # overlap (remainder) round: skip segs fully covered
                        # by the previous round
                        if j0 + seg_rows * (c + 1) <= new_from[j0]:
                            continue
                        eng = nc.sync if c % 2 == 0 else nc.scalar
                        lo_px = (j0 + seg_rows * c) * W
                        eng.dma_start(
                            out_v[g][:, :, lo_px : lo_px + NMM],
                            st[32 * c : 32 * c + 32, :, :],
                        )


def build_conv_p2(tc, out_ap, x_ap, w_ap, D, H, W, seg_rows, strip_rows):
    """64x64 tile variant: tile (rp, ct) contracts an image PAIR rp (block-
    diagonal 64x64 weights, 2 images x 32ch) for pixel segment ct. 4 matmuls
    per tap instead of 16; probes 2x column-tiling stream concurrency."""
    nc = tc.nc
    f32 = mybir.dt.float32
    bf16 = mybir.dt.bfloat16

    IPG = 4
    SEGS = 2                      # col tiles (64 wide)
    NRP = 2                       # row tiles = image pairs
    NMM = seg_rows * W
    RPR = SEGS * seg_rows         # 6 output rows per round
    assert D % IPG == 0
    NGRP = D // IPG
    WP = W + 2
    assert NMM <= 512

    origins = list(range(0, H - RPR + 1, RPR))
    covered = origins[-1] + RPR
    if covered < H:
        origins.append(H - RPR)
    new_from = {}
    for k, j0 in enumerate(origins):
        new_from[j0] = max(j0, origins[k - 1] + RPR) if k else j0
    rounds_per_strip = max(1, strip_rows // RPR)
    strips = [
        origins[i : i + rounds_per_strip]
        for i in range(0, len(origins), rounds_per_strip)
    ]
    XROWS = max(js[-1] + RPR + 2 - js[0] for js in strips)

    x_r = x_ap.rearrange("(g p) hp wp -> g p (hp wp)", g=NGRP, p=IPG * C)
    # img = 4g + 2*rp + ip; psum/st partitions carry (ip, o) in 32-blocks,
    # free carries rp. Out DMA per (ct, ip): src [32o, 2rp, px] matches
    # dst [o, rp(d-stride 2), px] with o outermost (16-way DGE split).
    out_v = out_ap.rearrange(
        "o (g rp ip) h w -> g ip o rp (h w)", g=NGRP, rp=2, ip=2
    )

    xbytes = XROWS * WP * 2
    xbufs = 3 if 3 * xbytes < 160 * 1024 else 2
    with (
        tc.tile_pool(name="wpool", bufs=1) as wpool,
        tc.tile_pool(name="xpool", bufs=xbufs) as xpool,
        tc.tile_pool(name="stpool", bufs=3) as stpool,
        tc.tile_pool(name="pspool", bufs=4, space="PSUM") as pspool,
    ):
        w_sb = wpool.tile([128, KH * KW, 64], bf16, tag="w")
        nc.sync.dma_start(w_sb[:], w_ap[:])

        for g in range(NGRP):
            for si, strip in enumerate(strips):
                X32 = xpool.tile([128, XROWS, WP], bf16, tag="X32")
                r0 = strip[0]
                nrows = strip[-1] + RPR + 2 - r0
                # ~1MB 4-round chunks; first strip leads with a 1-round chunk
                step = 4 * RPR
                cuts = ([0]
                        + [k * step + 2 for k in range(1, max(0, (nrows - 3))
                                                       // step + 1)]
                        + [nrows])
                if g == 0 and si == 0 and nrows > 3 * RPR:
                    cuts = sorted(set([RPR + 2] + cuts))
                cuts = sorted(set(c for c in cuts if c <= nrows))
                for a, b in zip(cuts, cuts[1:]):
                    nc.sync.dma_start(
                        X32[:, a:b, :].rearrange("p a b -> p (a b)"),
                        x_r[g][:, (r0 + a) * WP : (r0 + b) * WP],
                    )

                for j0 in strip:
                    ps = pspool.tile([128, NRP, 512], f32, tag="ps")
                    for tap in range(KH * KW):
                        kh, kw = divmod(tap, KW)
                        for ct in range(SEGS):
                            for rp in range(NRP):
                                j = j0 - r0 + seg_rows * ct
                                nc.tensor.matmul(
                                    ps[64 * ct : 64 * ct + 64, rp, 0:NMM],
                                    w_sb[64 * rp : 64 * rp + 64, tap, :],
                                    X32[
                                        64 * rp : 64 * rp + 64,
                                        j + kh : j + kh + seg_rows,
                                        kw : kw + W,
                                    ],
                                    start=(tap == 0),
                                    stop=(tap == KH * KW - 1),
                                    tile_position=(64 * rp, 64 * ct),
                                )
                    st = stpool.tile([128, NRP, NMM], bf16, tag="st")
                    nc.vector.tensor_copy(st[:, 0, :], ps[:, 0, 0:NMM])
                    nc.scalar.copy(st[:, 1, :], ps[:, 1, 0:NMM])
                    for ct in range(SEGS):
                        if j0 + seg_rows * (ct + 1) <= new_from[j0]:
                            continue
                        lo_px = (j0 + seg_rows * ct) * W
                        for ip in range(2):
                            eng = nc.sync if (ct + ip) % 2 == 0 else nc.scalar
                            eng.dma_start(
                                out_v[g][ip][:, :, lo_px : lo_px + NMM],
                                st[64 * ct + 32 * ip : 64 * ct + 32 * ip + 32,
                                   :, :],
                            )


def build_module(n_cores=8, D=FULL_D, H=FULL_H, W=FULL_W, seg_rows=3,
                 strip_rows=None, variant="f32"):
    if strip_rows is None:
        strip_rows = 36 if variant == "bf16x3" else 96
    nc = bacc.Bacc(
        "TRN2",
        target_bir_lowering=False,
        debug=False,
        num_devices=n_cores,
    )
    bf_like = variant in ("bf16", "bf16w", "bf16p2", "bf16h")
    x_dram_dt = mybir.dt.bfloat16 if bf_like else mybir.dt.float32
    x_d = nc.dram_tensor(
        "x", [D * C, H + 2, W + 2], x_dram_dt, kind="ExternalInput"
    )
    if variant == "f32":
        w_d = nc.dram_tensor(
            "w", [128, KH * KW * C], mybir.dt.float32, kind="ExternalInput"
        )
    elif variant in ("bf16", "bf16h"):
        w_d = nc.dram_tensor(
            "w", [128, KH * KW * C], mybir.dt.bfloat16, kind="ExternalInput"
        )
    elif variant == "bf16w":
        w_d = nc.dram_tensor(
            "w", [128, KH * KW, 128], mybir.dt.bfloat16, kind="ExternalInput"
        )
    elif variant == "bf16p2":
        w_d = nc.dram_tensor(
            "w", [128, KH * KW, 64], mybir.dt.bfloat16, kind="ExternalInput"
        )
    else:
        w_d = nc.dram_tensor(
            "w", [128, 2, KH * KW * C], mybir.dt.bfloat16, kind="ExternalInput"
        )
    out_dram_dt = mybir.dt.bfloat16 if bf_like else mybir.dt.float32
    out_d = nc.dram_tensor(
        "out", [C, D, H, W], out_dram_dt, kind="ExternalOutput"
    )
    with tile.TileContext(nc) as tc:
        if variant == "bf16p2":
            build_conv_p2(
                tc, out_d.ap(), x_d.ap(), w_d.ap(), D, H, W, seg_rows,
                strip_rows,
            )
        else:
            build_conv(
                tc, out_d.ap(), x_d.ap(), w_d.ap(), D, H, W, seg_rows,
                strip_rows, variant,
            )
    nc.compile()
    return nc


def binarize_weights(weights, variant="bf16x3"):
    """Host-side: [32,32,1,3,3] fp32 -> packed replicated weight tile.
    w_packed[32r+i, 32*tap+o] = scale[o,i] * sign(w[o,i,kh,kw]), tap = kh*3+kw.
    f32: [128, 288] f32.  bf16x3: [128, 2, 288] bf16 (hi, lo split)."""
    w = np.asarray(weights, dtype=np.float32)
    scale = np.mean(np.abs(w), axis=(2, 3, 4), keepdims=True)
    bw = (scale * np.sign(w)).astype(np.float32)          # [o, i, 1, 3, 3]
    wt = bw[:, :, 0].transpose(1, 2, 3, 0).reshape(C, KH * KW * C)  # [i, tap*32+o]
    full = np.ascontiguousarray(np.tile(wt, (4, 1)))       # [128, 288] f32
    if variant == "f32":
        return full
    import ml_dtypes
    if variant in ("bf16", "bf16h"):
        return np.ascontiguousarray(full.astype(ml_dtypes.bfloat16))
    if variant == "bf16w":
        # [128, 9, 128]: sub-array (r, c) holds the same 32x32 tap block
        w9 = wt.reshape(C, KH * KW, C)                     # [i, tap, o]
        w_full = np.tile(w9, (4, 1, 4))                    # [128, 9, 128]
        return np.ascontiguousarray(w_full.astype(ml_dtypes.bfloat16))
    if variant == "bf16p2":
        # [128, 9, 64]: block-diagonal 2-image pair weights per 64-row tile.
        # Output col index = 32*ip + o (ip-major 32-blocks).
        w9 = wt.reshape(C, KH * KW, C)                     # [i, tap, o]
        w64 = np.zeros((64, KH * KW, 64), dtype=np.float32)
        w64[0:32, :, 0:32] = w9
        w64[32:64, :, 32:64] = w9
        return np.ascontiguousarray(
            np.tile(w64, (2, 1, 1)).astype(ml_dtypes.bfloat16))
    hi = full.astype(ml_dtypes.bfloat16)
    lo = (full - hi.astype(np.float32)).astype(ml_dtypes.bfloat16)
    return np.ascontiguousarray(np.stack([hi, lo], axis=1))  # [128, 2, 288] bf16


_NC_CACHE = {}


def _get_nc(key, **kwargs):
    if key not in _NC_CACHE:
        _NC_CACHE[key] = build_module(**kwargs)
    return _NC_CACHE[key]


def pad_input(x, dtype=np.float32):
    """[N, C, D, H, W] f32 -> [N, D*C, H+2, W+2] zero-padded, d-major."""
    n, c, d, h, w = x.shape
    xp = np.zeros((n, d, c, h + 2, w + 2), dtype=dtype)
    xp[:, :, :, 1 : h + 1, 1 : w + 1] = x.transpose(0, 2, 1, 3, 4)
    return xp.reshape(n, d * c, h + 2, w + 2)


def run(x, weights, trace=False, variant="bf16", seg_rows=3, strip_rows=None):
    x = np.asarray(x, dtype=np.float32)
    n_cores = x.shape[0]
    key = (n_cores, variant, seg_rows, strip_rows)
    nc = _get_nc(
        key, n_cores=n_cores, seg_rows=seg_rows, strip_rows=strip_rows,
        variant=variant,
    )
    if variant in ("bf16", "bf16w", "bf16p2", "bf16h"):
        import ml_dtypes
        xp = pad_input(x, dtype=ml_dtypes.bfloat16)
    else:
        xp = pad_input(x)
    w_packed = binarize_weights(weights, variant)
    in_maps = [{"x": xp[n], "w": w_packed} for n in range(n_cores)]
    res = bass_utils.run_bass_kernel_spmd(
        nc, in_maps, core_ids=list(range(n_cores)), trace=trace
    )
    out = np.stack([res.results[n]["out"] for n in range(n_cores)])
    if out.dtype != np.float32:
        out = out.astype(np.float32)
    return out, res


def kernel(x, weights):
    out, _ = run(x, weights)
    return out



# revision 54
# speedup vs baseline: 1.8830x; 1.8830x over previous
"""Binary-weight 3x3 conv (depth-1 conv3d), 32ch -> 32ch, on trn2.

Forward pass of a BNN conv: effective weights are scale[o,i] * sign(w[o,i,kh,kw])
(the straight-through-estimator machinery in the reference only affects grads).
Kernel depth is 1, so this is a 2D 3x3 same-padded conv applied independently to
each of N*D = 8*16 = 128 images of shape [32, 160, 160].

Strategy (per core; batch dim sharded 1:1 onto 8 cores):
  - 16 d-slices per core, processed in 4 groups of 4 images.
  - Images live in SBUF zero-padded to 162 cols (host pre-pads + casts to
    bf16; rel-err budget 2e-2 >> bf16 conv error ~4e-3), on 32 channels =
    partitions [32r, 32r+32) for image r of the group.
  - Default "bf16p2": PE runs 64x64 tiles — tile (rp, ct) contracts an image
    PAIR rp (block-diagonal 2x32ch weights) for pixel segment ct; 4 matmuls
    per tap, 9 taps accumulate in PSUM (tap shifts = free-axis AP offsets).
    Measured best vs the 32x32 16-tile packing ("bf16", ~330us): halved
    instruction stream wins ~17us. The PE moving path streams ~one
    128-partition column/cycle, which floors any tap-streamed carving here.
  - PSUM evacuated to SBUF as bf16 split between DVE and ACT (halves the
    out DMA), then one DMA per seg writes into the NCDHW output layout;
    host casts the bf16 result back to f32.
"""

import numpy as np

import concourse.bass as bass
import concourse.mybir as mybir
import concourse.tile as tile
from concourse import bacc
from concourse import bass_utils
from concourse.tile_rust import add_dep_helper

C = 32          # in = out channels
KH = KW = 3

# full-problem dims
FULL_N, FULL_D, FULL_H, FULL_W = 8, 16, 160, 160


def build_conv(tc, out_ap, x_ap, w_ap, D, H, W, seg_rows, strip_rows, variant):
    """Emit the conv program for one core. x: [32, D, H, W], out: [32, D, H, W].

    variant "f32":   exact fp32 matmuls (4 cyc/row), w: [128, 288] f32.
    variant "bf16x3": x and w each split into bf16 hi+lo; accumulate
        w_hi*x_hi + w_hi*x_lo + w_lo*x_hi (error ~2^-18), w: [128, 2, 288] bf16.
    """
    nc = tc.nc
    f32 = mybir.dt.float32
    bf16 = mybir.dt.bfloat16
    mm_dt = f32 if variant == "f32" else bf16
    bf_like = variant in ("bf16", "bf16w", "bf16h")
    shared_w = variant == "bf16w"
    hsplit = variant == "bf16h"
    x_dt = bf16 if bf_like else f32
    st_dt = bf16 if bf_like else f32

    IPG = 4                      # images per group (row tiles)
    SEGS = 4                     # pixel segments per round (col tiles)
    NMM = seg_rows * W           # moving free size per matmul
    RPR = SEGS * seg_rows        # output rows per round
    assert D % IPG == 0
    NGRP = D // IPG
    WP = W + 2
    assert NMM <= 512

    # round origins (first output row of each round); if RPR doesn't divide
    # H, a final overlap round recomputes a few rows (identical values)
    origins = list(range(0, H - RPR + 1, RPR))
    covered = origins[-1] + RPR
    if covered < H:
        origins.append(H - RPR)
    new_from = {}
    for k, j0 in enumerate(origins):
        new_from[j0] = max(j0, origins[k - 1] + RPR) if k else j0
    rounds_per_strip = max(1, strip_rows // RPR)
    strips = [
        origins[i : i + rounds_per_strip]
        for i in range(0, len(origins), rounds_per_strip)
    ]
    XROWS = max(js[-1] + RPR + 2 - js[0] for js in strips)

    # x_ap is host-prepadded: [D, C, H+2, W+2] with zero borders, so a strip
    # is one fully-contiguous DMA per partition (partition stride = (H+2)*(W+2)).
    x_r = x_ap.rearrange("(g p) hp wp -> g p (hp wp)", g=NGRP, p=IPG * C)
    # [g] -> (o, r, h*w): row-sliced per round/seg at DMA time. o outermost
    # so the DGE splits each output DMA across all 16 SDMA engines (it
    # splits on the outermost dest dim; with 4 outermost it used only 4)
    out_v = out_ap.rearrange("o (g r) h w -> g o r (h w)", g=NGRP, r=IPG)

    xbytes = XROWS * WP * (2 if bf_like else 4)
    xbufs = 3 if (variant != "bf16x3" and 3 * xbytes < 160 * 1024) else 2
    with (
        tc.tile_pool(name="wpool", bufs=1) as wpool,
        tc.tile_pool(name="xpool", bufs=xbufs) as xpool,
        tc.tile_pool(name="stpool", bufs=3) as stpool,
        tc.tile_pool(name="pspool", bufs=2, space="PSUM") as pspool,
    ):
        if variant == "f32":
            w_sb = wpool.tile([128, KH * KW * C], f32, tag="w")
        elif variant in ("bf16", "bf16h"):
            w_sb = wpool.tile([128, KH * KW * C], bf16, tag="w")
        elif variant == "bf16w":
            w_sb = wpool.tile([128, KH * KW, 128], bf16, tag="w")
        else:
            w_sb = wpool.tile([128, 2, KH * KW * C], bf16, tag="w")
        nc.sync.dma_start(w_sb[:], w_ap[:])
        prev_mms = []  # tensor-engine MMs the next array reload must follow
        prev_h1 = {}   # bf16h: last weight-reusing MM per tile position

        for g in range(NGRP):
            for si, strip in enumerate(strips):
                X32 = xpool.tile([128, XROWS, WP], x_dt, tag="X32")
                r0 = strip[0]
                nrows = strip[-1] + RPR + 2 - r0
                # padded rows [r0, r0+nrows) of each image, contiguous runs
                # per partition. The very first strip is split into chunks so
                # the first rounds' matmuls start ~20us earlier.
                # chunk strip DMAs into ~2-round (~1MB) pieces so rounds wait
                # on partial arrivals and HBM load is smoothed; the first
                # strip's first chunk is one round so matmuls start early
                step = 2 * RPR
                cuts = ([0]
                        + [k * step + 2 for k in range(1, max(0, (nrows - 3))
                                                       // step + 1)]
                        + [nrows])
                if g == 0 and si == 0 and nrows > 3 * RPR:
                    cuts = sorted(set([RPR + 2] + cuts))
                cuts = sorted(set(c for c in cuts if c <= nrows))
                for a, b in zip(cuts, cuts[1:]):
                    nc.sync.dma_start(
                        X32[:, a:b, :].rearrange("p a b -> p (a b)"),
                        x_r[g][:, (r0 + a) * WP : (r0 + b) * WP],
                    )

                if variant in ("f32", "bf16", "bf16w", "bf16h"):
                    # comp -> (weight slice index or None, moving buffer)
                    comps = [(None, X32)]
                else:
                    Xhi = xpool.tile([128, XROWS, WP], bf16, tag="Xhi")
                    Xlo = xpool.tile([128, XROWS, WP], bf16, tag="Xlo")
                    nc.scalar.copy(Xhi[:, 0:nrows, :], X32[:, 0:nrows, :])
                    nc.vector.tensor_sub(
                        Xlo[:, 0:nrows, :], X32[:, 0:nrows, :],
                        Xhi[:, 0:nrows, :],
                    )
                    comps = [(0, Xhi), (0, Xlo), (1, Xhi)]

                for j0 in strip:
                    # split the round's accumulators into two 2-bank tiles so
                    # each half releases as soon as its own evac copy is done
                    psA = pspool.tile([128, 2, 512], f32, tag="psA")
                    psB = pspool.tile([128, 2, 512], f32, tag="psB")
                    for ci, (wi, XB) in enumerate(comps):
                        for tap in range(KH * KW):
                            kh, kw = divmod(tap, KW)
                            if shared_w:
                                # one 128-col array load for all 16 tiles of
                                # this tap (w replicated 4x4 host-side)
                                ldw = nc.tensor.ldweights(w_sb[:, tap, :])
                                for m in prev_mms:
                                    add_dep_helper(
                                        ldw.ins, m.ins, sync=False,
                                        reason="array reload after prior tap",
                                    )
                                prev_mms = []
                            if hsplit:
                                # split each tile matmul into a 1-row (h0,
                                # carries the LDW + bank-clear) and a 2-row
                                # (h1, ldweights=False) piece, h0s of all
                                # tiles first: doubles the per-lane tile
                                # handoffs whose fill/drain overlap (~100cyc
                                # each) shortens the serialized stream
                                hmms = {}
                                for h in (0, 1):
                                    for c in range(SEGS):
                                        for r in range(IPG):
                                            lhsT = w_sb[
                                                32 * r : 32 * r + 32,
                                                32 * tap : 32 * tap + 32,
                                            ]
                                            j = j0 - r0 + seg_rows * c
                                            psh = psA if r < 2 else psB
                                            if h == 0:
                                                mm = nc.tensor.matmul(
                                                    psh[32 * c : 32 * c + 32,
                                                        r % 2, 0:W],
                                                    lhsT,
                                                    XB[32 * r : 32 * r + 32,
                                                       j + kh : j + kh + 1,
                                                       kw : kw + W],
                                                    start=(tap == 0),
                                                    stop=(tap == KH * KW - 1),
                                                    tile_position=(32 * r,
                                                                   32 * c),
                                                )
                                                prev = prev_h1.pop((c, r), None)
                                                if prev is not None:
                                                    add_dep_helper(
                                                        mm.ins, prev.ins,
                                                        sync=False,
                                                        reason="h1 before reload",
                                                    )
                                                hmms[(c, r)] = mm
                                            else:
                                                mm = nc.tensor.matmul(
                                                    psh[32 * c : 32 * c + 32,
                                                        r % 2, W:NMM],
                                                    lhsT,
                                                    XB[32 * r : 32 * r + 32,
                                                       j + kh + 1 :
                                                       j + kh + seg_rows,
                                                       kw : kw + W],
                                                    start=False,
                                                    stop=(tap == KH * KW - 1),
                                                    tile_position=(32 * r,
                                                                   32 * c),
                                                    skip_group_check=True,
                                                )
                                                mm.ins.ldweights = False
                                                add_dep_helper(
                                                    mm.ins,
                                                    hmms[(c, r)].ins,
                                                    sync=False,
                                                    reason="reuse h0 weights",
                                                )
                                                prev_h1[(c, r)] = mm
                                continue
                            # r innermost: consecutive matmuls hit different
                            # PE row groups so their weight loads overlap
                            for c in range(SEGS):
                                for r in range(IPG):
                                    if shared_w:
                                        lhsT = w_sb[
                                            32 * r : 32 * r + 32, tap,
                                            32 * c : 32 * c + 32,
                                        ]
                                    elif wi is None:
                                        lhsT = w_sb[
                                            32 * r : 32 * r + 32,
                                            32 * tap : 32 * tap + 32,
                                        ]
                                    else:
                                        lhsT = w_sb[
                                            32 * r : 32 * r + 32, wi,
                                            32 * tap : 32 * tap + 32,
                                        ]
                                    j = j0 - r0 + seg_rows * c
                                    rhs = XB[
                                        32 * r : 32 * r + 32,
                                        j + kh : j + kh + seg_rows,
                                        kw : kw + W,
                                    ]
                                    psh = psA if r < 2 else psB
                                    mm = nc.tensor.matmul(
                                        psh[32 * c : 32 * c + 32, r % 2, 0:NMM],
                                        lhsT,
                                        rhs,
                                        start=(ci == 0 and tap == 0),
                                        stop=(
                                            ci == len(comps) - 1
                                            and tap == KH * KW - 1
                                        ),
                                        tile_position=(32 * r, 32 * c),
                                    )
                                    if shared_w:
                                        mm.ins.ldweights = False
                                        add_dep_helper(
                                            mm.ins, ldw.ins, sync=False,
                                            reason="use tap weights",
                                        )
                                        prev_mms.append(mm)
                    st = stpool.tile([128, SEGS, NMM], st_dt, tag="st")
                    nc.vector.tensor_copy(st[:, 0:2, :], psA[:, :, 0:NMM])
                    nc.scalar.copy(st[:, 2:4, :], psB[:, :, 0:NMM])
                    for c in range(SEGS):
                        # overlap (remainder) round: skip segs fully covered
                        # by the previous round
                        if j0 + seg_rows * (c + 1) <= new_from[j0]:
                            continue
                        eng = nc.sync if c % 2 == 0 else nc.scalar
                        lo_px = (j0 + seg_rows * c) * W
                        eng.dma_start(
                            out_v[g][:, :, lo_px : lo_px + NMM],
                            st[32 * c : 32 * c + 32, :, :],
                        )


def build_conv_p2(tc, out_ap, x_ap, w_ap, D, H, W, seg_rows, strip_rows):
    """64x64 tile variant: tile (rp, ct) contracts an image PAIR rp (block-
    diagonal 64x64 weights, 2 images x 32ch) for pixel segment ct. 4 matmuls
    per tap instead of 16; probes 2x column-tiling stream concurrency."""
    nc = tc.nc
    f32 = mybir.dt.float32
    bf16 = mybir.dt.bfloat16

    IPG = 4
    SEGS = 2                      # col tiles (64 wide)
    NRP = 2                       # row tiles = image pairs
    NMM = seg_rows * W
    RPR = SEGS * seg_rows         # 6 output rows per round
    assert D % IPG == 0
    NGRP = D // IPG
    WP = W + 2
    assert NMM <= 512

    origins = list(range(0, H - RPR + 1, RPR))
    covered = origins[-1] + RPR
    if covered < H:
        origins.append(H - RPR)
    new_from = {}
    for k, j0 in enumerate(origins):
        new_from[j0] = max(j0, origins[k - 1] + RPR) if k else j0
    rounds_per_strip = max(1, strip_rows // RPR)
    strips = [
        origins[i : i + rounds_per_strip]
        for i in range(0, len(origins), rounds_per_strip)
    ]
    XROWS = max(js[-1] + RPR + 2 - js[0] for js in strips)

    x_r = x_ap.rearrange("(g p) hp wp -> g p (hp wp)", g=NGRP, p=IPG * C)
    # img = 4g + 2*rp + ip; psum/st partitions carry (ip, o) in 32-blocks,
    # free carries rp. Out DMA per (ct, ip): src [32o, 2rp, px] matches
    # dst [o, rp(d-stride 2), px] with o outermost (16-way DGE split).
    out_v = out_ap.rearrange(
        "o (g rp ip) h w -> g ip o rp (h w)", g=NGRP, rp=2, ip=2
    )

    xbytes = XROWS * WP * 2
    xbufs = 3 if 3 * xbytes < 160 * 1024 else 2
    with (
        tc.tile_pool(name="wpool", bufs=1) as wpool,
        tc.tile_pool(name="xpool", bufs=xbufs) as xpool,
        tc.tile_pool(name="stpool", bufs=3) as stpool,
        tc.tile_pool(name="pspool", bufs=4, space="PSUM") as pspool,
    ):
        w_sb = wpool.tile([128, KH * KW, 64], bf16, tag="w")
        nc.sync.dma_start(w_sb[:], w_ap[:])

        for g in range(NGRP):
            for si, strip in enumerate(strips):
                X32 = xpool.tile([128, XROWS, WP], bf16, tag="X32")
                r0 = strip[0]
                nrows = strip[-1] + RPR + 2 - r0
                # ~1MB 4-round chunks; first strip leads with a 1-round chunk
                step = 4 * RPR
                cuts = ([0]
                        + [k * step + 2 for k in range(1, max(0, (nrows - 3))
                                                       // step + 1)]
                        + [nrows])
                if g == 0 and si == 0 and nrows > 3 * RPR:
                    cuts = sorted(set([RPR + 2] + cuts))
                cuts = sorted(set(c for c in cuts if c <= nrows))
                for a, b in zip(cuts, cuts[1:]):
                    nc.sync.dma_start(
                        X32[:, a:b, :].rearrange("p a b -> p (a b)"),
                        x_r[g][:, (r0 + a) * WP : (r0 + b) * WP],
                    )

                for j0 in strip:
                    ps = pspool.tile([128, NRP, 512], f32, tag="ps")
                    for tap in range(KH * KW):
                        kh, kw = divmod(tap, KW)
                        for ct in range(SEGS):
                            for rp in range(NRP):
                                j = j0 - r0 + seg_rows * ct
                                nc.tensor.matmul(
                                    ps[64 * ct : 64 * ct + 64, rp, 0:NMM],
                                    w_sb[64 * rp : 64 * rp + 64, tap, :],
                                    X32[
                                        64 * rp : 64 * rp + 64,
                                        j + kh : j + kh + seg_rows,
                                        kw : kw + W,
                                    ],
                                    start=(tap == 0),
                                    stop=(tap == KH * KW - 1),
                                    tile_position=(64 * rp, 64 * ct),
                                )
                    st = stpool.tile([128, NRP, NMM], bf16, tag="st")
                    nc.vector.tensor_copy(st[:, 0, :], ps[:, 0, 0:NMM])
                    nc.scalar.copy(st[:, 1, :], ps[:, 1, 0:NMM])
                    for ct in range(SEGS):
                        if j0 + seg_rows * (ct + 1) <= new_from[j0]:
                            continue
                        lo_px = (j0 + seg_rows * ct) * W
                        for ip in range(2):
                            eng = nc.sync if (ct + ip) % 2 == 0 else nc.scalar
                            eng.dma_start(
                                out_v[g][ip][:, :, lo_px : lo_px + NMM],
                                st[64 * ct + 32 * ip : 64 * ct + 32 * ip + 32,
                                   :, :],
                            )


def build_module(n_cores=8, D=FULL_D, H=FULL_H, W=FULL_W, seg_rows=3,
                 strip_rows=None, variant="f32"):
    if strip_rows is None:
        strip_rows = 36 if variant == "bf16x3" else 96
    nc = bacc.Bacc(
        "TRN2",
        target_bir_lowering=False,
        debug=False,
        num_devices=n_cores,
    )
    bf_like = variant in ("bf16", "bf16w", "bf16p2", "bf16h")
    x_dram_dt = mybir.dt.bfloat16 if bf_like else mybir.dt.float32
    x_d = nc.dram_tensor(
        "x", [D * C, H + 2, W + 2], x_dram_dt, kind="ExternalInput"
    )
    if variant == "f32":
        w_d = nc.dram_tensor(
            "w", [128, KH * KW * C], mybir.dt.float32, kind="ExternalInput"
        )
    elif variant in ("bf16", "bf16h"):
        w_d = nc.dram_tensor(
            "w", [128, KH * KW * C], mybir.dt.bfloat16, kind="ExternalInput"
        )
    elif variant == "bf16w":
        w_d = nc.dram_tensor(
            "w", [128, KH * KW, 128], mybir.dt.bfloat16, kind="ExternalInput"
        )
    elif variant == "bf16p2":
        w_d = nc.dram_tensor(
            "w", [128, KH * KW, 64], mybir.dt.bfloat16, kind="ExternalInput"
        )
    else:
        w_d = nc.dram_tensor(
            "w", [128, 2, KH * KW * C], mybir.dt.bfloat16, kind="ExternalInput"
        )
    out_dram_dt = mybir.dt.bfloat16 if bf_like else mybir.dt.float32
    out_d = nc.dram_tensor(
        "out", [C, D, H, W], out_dram_dt, kind="ExternalOutput"
    )
    with tile.TileContext(nc) as tc:
        if variant == "bf16p2":
            build_conv_p2(
                tc, out_d.ap(), x_d.ap(), w_d.ap(), D, H, W, seg_rows,
                strip_rows,
            )
        else:
            build_conv(
                tc, out_d.ap(), x_d.ap(), w_d.ap(), D, H, W, seg_rows,
                strip_rows, variant,
            )
    nc.compile()
    return nc


def binarize_weights(weights, variant="bf16x3"):
    """Host-side: [32,32,1,3,3] fp32 -> packed replicated weight tile.
    w_packed[32r+i, 32*tap+o] = scale[o,i] * sign(w[o,i,kh,kw]), tap = kh*3+kw.
    f32: [128, 288] f32.  bf16x3: [128, 2, 288] bf16 (hi, lo split)."""
    w = np.asarray(weights, dtype=np.float32)
    scale = np.mean(np.abs(w), axis=(2, 3, 4), keepdims=True)
    bw = (scale * np.sign(w)).astype(np.float32)          # [o, i, 1, 3, 3]
    wt = bw[:, :, 0].transpose(1, 2, 3, 0).reshape(C, KH * KW * C)  # [i, tap*32+o]
    full = np.ascontiguousarray(np.tile(wt, (4, 1)))       # [128, 288] f32
    if variant == "f32":
        return full
    import ml_dtypes
    if variant in ("bf16", "bf16h"):
        return np.ascontiguousarray(full.astype(ml_dtypes.bfloat16))
    if variant == "bf16w":
        # [128, 9, 128]: sub-array (r, c) holds the same 32x32 tap block
        w9 = wt.reshape(C, KH * KW, C)                     # [i, tap, o]
        w_full = np.tile(w9, (4, 1, 4))                    # [128, 9, 128]
        return np.ascontiguousarray(w_full.astype(ml_dtypes.bfloat16))
    if variant == "bf16p2":
        # [128, 9, 64]: block-diagonal 2-image pair weights per 64-row tile.
        # Output col index = 32*ip + o (ip-major 32-blocks).
        w9 = wt.reshape(C, KH * KW, C)                     # [i, tap, o]
        w64 = np.zeros((64, KH * KW, 64), dtype=np.float32)
        w64[0:32, :, 0:32] = w9
        w64[32:64, :, 32:64] = w9
        return np.ascontiguousarray(
            np.tile(w64, (2, 1, 1)).astype(ml_dtypes.bfloat16))
    hi = full.astype(ml_dtypes.bfloat16)
    lo = (full - hi.astype(np.float32)).astype(ml_dtypes.bfloat16)
    return np.ascontiguousarray(np.stack([hi, lo], axis=1))  # [128, 2, 288] bf16


_NC_CACHE = {}


def _get_nc(key, **kwargs):
    if key not in _NC_CACHE:
        _NC_CACHE[key] = build_module(**kwargs)
    return _NC_CACHE[key]


def pad_input(x, dtype=np.float32):
    """[N, C, D, H, W] f32 -> [N, D*C, H+2, W+2] zero-padded, d-major."""
    n, c, d, h, w = x.shape
    xp = np.zeros((n, d, c, h + 2, w + 2), dtype=dtype)
    xp[:, :, :, 1 : h + 1, 1 : w + 1] = x.transpose(0, 2, 1, 3, 4)
    return xp.reshape(n, d * c, h + 2, w + 2)


def run(x, weights, trace=False, variant="bf16p2", seg_rows=3, strip_rows=None):
    x = np.asarray(x, dtype=np.float32)
    n_cores = x.shape[0]
    key = (n_cores, variant, seg_rows, strip_rows)
    nc = _get_nc(
        key, n_cores=n_cores, seg_rows=seg_rows, strip_rows=strip_rows,
        variant=variant,
    )
    if variant in ("bf16", "bf16w", "bf16p2", "bf16h"):
        import ml_dtypes
        xp = pad_input(x, dtype=ml_dtypes.bfloat16)
    else:
        xp = pad_input(x)
    w_packed = binarize_weights(weights, variant)
    in_maps = [{"x": xp[n], "w": w_packed} for n in range(n_cores)]
    res = bass_utils.run_bass_kernel_spmd(
        nc, in_maps, core_ids=list(range(n_cores)), trace=trace
    )
    out = np.stack([res.results[n]["out"] for n in range(n_cores)])
    if out.dtype != np.float32:
        out = out.astype(np.float32)
    return out, res


def kernel(x, weights):
    out, _ = run(x, weights)
    return out

